# revision 45
# baseline (speedup 1.0000x reference)
"""Trainium2 Bass kernel for DecoderRNNWithAttention (teacher-forced LSTM decoder).

Key mathematical simplification: the attention block is an exact no-op.
The encoder output has a single spatial position, so softmax over that
axis is exactly 1.0 and context == features, independent of h. Hence:
  - the enc/dec/full attention projections never affect the output;
  - the input-side gate contributions Gx = X @ W_ih.T + (b_ih + b_hh)
    can be precomputed for all T steps in one batched matmul
    (X_t = [word_t ; features]);
  - the serial recurrence is only gates_t = Gx_t + h_t @ W_hh.T plus the
    LSTM elementwise cell; logits_t = h_{t+1} @ fcn_W.T + fcn_b.

Sharding: pure data-parallel over batch. 8 cores x 16 rows, no collectives.

Device layouts (all "transposed" so the partition dim is the feature dim):
  - gate dim 4H split into 32 slices of 128, permuted [i f o g] so one
    sigmoid covers cols 0:384 and one tanh covers cols 384:512 of the
    per-step [128, 512] gate tile (cols = slice-block * 16 batch).
  - h state history Hall[128, t*128 + k*16 + b] (k = H-tile), written
    once per step as one [128, 128] tile; doubles as matmul rhs slices.
  - vocab projection: out.T tiles [V-tile 128, 384 rows], rows = (t, b).
"""

import numpy as np
import ml_dtypes

import concourse.bacc as bacc
import concourse.mybir as mybir
import concourse.tile as tile
from concourse.bass_utils import run_bass_kernel_spmd

B, T, E, H, V, ENC = 128, 25, 512, 1024, 32000, 400
NCORES = 8
BS = B // NCORES          # 16 batch rows per core
TB = T * BS               # 400 = matmul N for phase 1
ROWS = (T - 1) * BS       # 384 = matmul N for the vocab projection
KT = H // 128             # 8 K-tiles
GS = 4 * H // 128         # 32 gate slices
VT = V // 128             # 250 vocab tiles
NCH = 63                  # fcn weight chunks (512 vocab cols each)
VP = NCH * 512            # 32256 = vocab padded to whole chunks
VTP = VP // 128           # 252 padded vocab tiles
XDIM = E + ENC            # 912, padded to 1024

# torch LSTMCell gate order is [i f g o] — kept as-is: the o-gate stream
# lands last so the c-path (needs i,f,g) overlaps the o matmuls, and the
# per-group adds/activations pipeline with the stream.
PERM_SRC = list(range(0, 32))

# dtype config: 'f32' exact, 'f32r' tf32-like at bf16 speed (N>=256),
# 'bf16' fastest. Storage for f32r is fp32 + bitcast at the matmul.
CFG = {
    "p1": "bf16",    # phase-1 (Gx) matmul dtype
    "rec": "f8",     # recurrence (W_hh) matmul dtype (fp8 e4m3, x256 prescale)
    "fcn": "bf16",   # vocab projection matmul dtype
    "vchunk": 512,  # fcn weight streaming chunk (vocab cols per DMA)
    "jn": 64,        # recurrence matmul free dim (>=BS; junk cols keep HAM warm)
    "npf": 4,        # fcn weight chunks prefetched into SBUF during P2
    "og": 9,         # fcn chunks per grouped output DMA (63 = 7 * 9)
    "odt": "f16",    # output store dtype (f16 halves the logits DMA bytes)
    "dr": 0,         # fp8 DoubleRow recurrence (measured net loss; keep off)
}

_F32 = mybir.dt.float32
_DT = {"f32": mybir.dt.float32, "f32r": mybir.dt.float32r, "bf16": mybir.dt.bfloat16,
       "f16": mybir.dt.float16, "f8": mybir.dt.float8e4}
_NPDT = {"f32": np.float32, "f32r": np.float32, "bf16": ml_dtypes.bfloat16,
         "f16": np.float16, "f8": ml_dtypes.float8_e4m3}
# fp8 W_hh is pre-scaled so ~all weights land in e4m3's normal range; the
# inverse scale folds into the gate activations (out = f(in * scale)).
_RSC = 256.0


def _mm_ap(ap, kind):
    """Tiles carrying f32r data are declared float32r; nothing to do."""
    return ap


def build_nc(cfg=CFG):
    AF = mybir.ActivationFunctionType
    p1, rec, fcn = cfg["p1"], cfg["rec"], cfg["fcn"]
    VC = cfg["vchunk"]
    odt = cfg.get("odt", "f16")
    NPF = cfg.get("npf", 8)
    OG = cfg.get("og", 9)
    NOG = NCH // OG
    assert NOG * OG == NCH
    # gates carry an extra factor RSC when W_hh is fp8-prescaled
    RSC = _RSC if rec == "f8" else 1.0
    DR = bool(cfg.get("dr", 0)) and rec == "f8"   # fp8 DoubleRow recurrence
    hdt = ("f8" if DR else "bf16") if rec == "f8" else rec  # h storage dtype

    JN = cfg.get("jn", BS)

    nc = bacc.Bacc()
    xT_d = nc.dram_tensor("xT", [128, KT * TB], _DT[p1], kind="ExternalInput")
    wih_d = nc.dram_tensor("wih", [128, KT * 4 * H], _DT[p1], kind="ExternalInput")
    whh_d = nc.dram_tensor("whh", [128, KT * 4 * H], _DT[rec], kind="ExternalInput")
    fcnw_d = nc.dram_tensor("fcnw", [NCH, 128, KT * 512], _DT[fcn], kind="ExternalInput")
    bsum_d = nc.dram_tensor("bsum", [128, GS], _F32, kind="ExternalInput")
    fb_d = nc.dram_tensor("fb", [128, VTP], _F32, kind="ExternalInput")
    out_d = nc.dram_tensor("out", [128, NCH * 4 * ROWS], _DT[odt],
                           kind="ExternalOutput")

    with tile.TileContext(nc) as tc:
        with (
            tc.tile_pool(name="pers", bufs=1) as pers,
            tc.tile_pool(name="psum", bufs=4, space="PSUM") as psum,
            tc.tile_pool(name="elem", bufs=2) as elem,
        ):
            hall = pers.tile([128, T * 128], _DT[hdt])
            xt_sb = pers.tile([128, KT * TB], _DT[p1])
            bsum_sb = pers.tile([128, GS], _F32)
            fb_sb = pers.tile([128, VTP], _F32)
            # fcn weight prefetch pool: allocated BELOW whhp/gxtp on the SBUF
            # stack so it survives their (LIFO) release into phase 3. The
            # prefetch DMAs depend only on DRAM, so they drain during the
            # recurrence while the DMA queues are otherwise idle.
            pfp = tc.alloc_tile_pool(name="pfp", bufs=1)
            pf = [pfp.tile([128, KT * 512], _DT[fcn], name=f"pf{c}")
                  for c in range(NPF)]
            # W_hh and Gx live only through the recurrence; own pools so the
            # space can be released to the fcn weight-streaming pool (LIFO).
            whhp = tc.alloc_tile_pool(name="whhp", bufs=1)
            gxtp = tc.alloc_tile_pool(name="gxtp", bufs=1)
            whh_sb = whhp.tile([128, KT * 4 * H], _DT[rec], name="whh_sb")
            gxt = gxtp.tile([128, GS * TB], _F32, name="gxt")

            nc.sync.dma_start(xt_sb[:], xT_d[:])
            nc.sync.dma_start(bsum_sb[:], bsum_d[:])
            nc.sync.dma_start(fb_sb[:], fb_d[:])
            nc.gpsimd.memset(hall[:], 0.0)

            # ---------------- Phase 1: Gx = X @ W_ih.T + (b_ih + b_hh) ----
            with tc.tile_pool(name="wihp", bufs=2) as wihp:
                for quarter in range(4):
                    wih_sb = wihp.tile([128, KT * 1024], _DT[p1], tag="wih")
                    for k in range(KT):
                        nc.sync.dma_start(
                            wih_sb[:, k * 1024:(k + 1) * 1024],
                            wih_d[:, k * 4096 + quarter * 1024:
                                  k * 4096 + quarter * 1024 + 1024])
                    for jj in range(8):
                        j = quarter * 8 + jj
                        ps = psum.tile([128, TB], _F32, tag="ps", name="ps", bufs=2)
                        for k in range(KT):
                            nc.tensor.matmul(
                                ps[:],
                                _mm_ap(wih_sb[:, k * 1024 + jj * 128:
                                              k * 1024 + jj * 128 + 128], p1),
                                _mm_ap(xt_sb[:, k * TB:(k + 1) * TB], p1),
                                start=(k == 0), stop=(k == KT - 1))
                        nc.scalar.activation(
                            gxt[:, j * TB:(j + 1) * TB], ps[:], AF.Identity,
                            bias=bsum_sb[:, j:j + 1], scale=RSC)

            # W_hh load ordered after phase-1 inputs so phase 1 starts early
            for k in range(KT):
                nc.sync.dma_start(whh_sb[:, k * 4096:(k + 1) * 4096],
                                  whh_d[:, k * 4096:(k + 1) * 4096])
            for c in range(NPF):
                # dummy write gives the prefetch DMA a dep on the last P1
                # activation, keeping its HBM traffic out of P1's window
                # (it drains during the recurrence instead)
                nc.vector.tensor_copy(pf[c][:, 0:1], gxt[:, GS * TB - 1:GS * TB])
                nc.sync.dma_start(pf[c][:], fcnw_d[c])

            # ---------------- Phase 2: LSTM recurrence --------------------
            # gxt viewed as [128, slice j, t, b]
            gxt_r = gxt.rearrange("p (j t b) -> p j (t b)", j=GS, t=T, b=BS)

            c_prev = None
            for t in range(T):
                if t == 0:
                    # gate order [i f g o]; c0 == 0 so f is unused
                    gates_src = gxt_r[:, :, 0:BS]  # [128, 32, 16] strided
                    sig_sb = elem.tile([128, 8, BS], _F32, tag="sig0", name="sg0")
                    nc.scalar.activation(sig_sb[:], gates_src[:, 0:8, :], AF.Sigmoid,
                                         scale=1.0 / RSC)
                    tg = elem.tile([128, 8, BS], _F32, tag="tg", name="tg")
                    nc.scalar.activation(tg[:], gates_src[:, 16:24, :], AF.Tanh,
                                         scale=1.0 / RSC)
                    sigo = elem.tile([128, 8, BS], _F32, tag="sigo", name="so0")
                    nc.scalar.activation(sigo[:], gates_src[:, 24:32, :], AF.Sigmoid,
                                         scale=1.0 / RSC)
                    sigo2 = sigo.rearrange("p a b -> p (a b)")
                    cn = elem.tile([128, 128], _F32, tag="c", name="cn")
                    nc.vector.tensor_mul(cn[:], sig_sb.rearrange("p a b -> p (a b)"),
                                         tg.rearrange("p a b -> p (a b)"))
                else:
                    # separate PSUM tiles per gate group so the adds/ACTs
                    # depend only on their own group's matmuls and pipeline
                    # with the still-running stream (i,f first, o last)
                    ps_if = psum.tile([128, 16 * JN], _F32, tag="psif",
                                      name="psif", bufs=1 if JN > 32 else 2)
                    ps_gg = psum.tile([128, 8 * JN], _F32, tag="psgg",
                                      name="psgg", bufs=1 if JN > 32 else 2)
                    ps_oa = psum.tile([128, 4 * JN], _F32, tag="psoa",
                                      name="psoa", bufs=1)
                    ps_ob = psum.tile([128, 4 * JN], _F32, tag="psob",
                                      name="psob", bufs=1)

                    if DR:
                        def mmgroup(pst, j0, j1):
                            # fp8 DoubleRow: one matmul contracts a k-tile
                            # PAIR (256 rows); lhsT [128, 2, 128] interleaved
                            # on host, rhs [128, 2, JN] is just a strided view
                            # of hall's (k,b) column layout.
                            for j in range(j0, j1):
                                for a in range(KT // 2):
                                    nc.tensor.matmul(
                                        pst[:, (j - j0) * JN:(j - j0) * JN + JN],
                                        whh_sb[:, a * 8192:(a + 1) * 8192]
                                        .rearrange("p (o g) -> p o g", o=2)
                                        [:, :, j * 128:(j + 1) * 128],
                                        hall[:, (t - 1) * 128 + a * 32:
                                             (t - 1) * 128 + a * 32 + 2 * JN]
                                        .rearrange("p (o n) -> p o n", o=2),
                                        start=(a == 0), stop=(a == KT // 2 - 1),
                                        perf_mode=mybir.MatmulPerfMode.DoubleRow)
                    else:
                        def mmgroup(pst, j0, j1):
                            for j in range(j0, j1):
                                for k in range(KT):
                                    nc.tensor.matmul(
                                        pst[:, (j - j0) * JN:(j - j0) * JN + JN],
                                        _mm_ap(whh_sb[:, k * 4096 + j * 128:
                                                      k * 4096 + j * 128 + 128], rec),
                                        _mm_ap(hall[:, (t - 1) * 128 + k * BS:
                                                    (t - 1) * 128 + k * BS + JN], rec),
                                        start=(k == 0), stop=(k == KT - 1))

                    gates_sb = elem.tile([128, GS, BS], _F32, tag="gates", name="gts")
                    g2 = gates_sb.rearrange("p a b -> p (a b)")

                    mmgroup(ps_if, 0, 16)
                    nc.vector.tensor_add(
                        gates_sb[:, 0:16, :],
                        ps_if.rearrange("p (j n) -> p j n", n=JN)[:, :, 0:BS],
                        gxt_r[:, 0:16, t * BS:(t + 1) * BS])
                    sig_sb = elem.tile([128, 256], _F32, tag="sig", name="sig")
                    nc.scalar.activation(sig_sb[:], g2[:, 0:256], AF.Sigmoid,
                                         scale=1.0 / RSC)

                    mmgroup(ps_gg, 16, 24)
                    nc.vector.tensor_add(
                        gates_sb[:, 16:24, :],
                        ps_gg.rearrange("p (j n) -> p j n", n=JN)[:, :, 0:BS],
                        gxt_r[:, 16:24, t * BS:(t + 1) * BS])
                    tg = elem.tile([128, 128], _F32, tag="tg", name="tg")
                    nc.scalar.activation(tg[:], g2[:, 256:384], AF.Tanh,
                                         scale=1.0 / RSC)
                    cn = elem.tile([128, 128], _F32, tag="c", name="cn")
                    nc.vector.tensor_mul(cn[:], sig_sb[:, 128:256], c_prev[:])
                    t1 = elem.tile([128, 128], _F32, tag="t1", name="t1")
                    nc.vector.tensor_mul(t1[:], sig_sb[:, 0:128], tg[:])
                    nc.vector.tensor_add(cn[:], cn[:], t1[:])
                    # thc queued before sig_o on the (FIFO) scalar engine: its
                    # input is ready well before the o-gate matmuls finish
                    thc = elem.tile([128, 128], _F32, tag="thc", name="thc")
                    nc.scalar.activation(thc[:], cn[:], AF.Tanh)

                    # o-group in two halves: half A's add/sigmoid/h-write hide
                    # under half B's matmul stream, shortening the serial tail
                    sigo = elem.tile([128, 128], _F32, tag="sigo", name="sgo")
                    mmgroup(ps_oa, 24, 28)
                    nc.vector.tensor_add(
                        gates_sb[:, 24:28, :],
                        ps_oa.rearrange("p (j n) -> p j n", n=JN)[:, :, 0:BS],
                        gxt_r[:, 24:28, t * BS:(t + 1) * BS])
                    nc.scalar.activation(sigo[:, 0:64], g2[:, 384:448], AF.Sigmoid,
                                         scale=1.0 / RSC)
                    nc.vector.tensor_mul(hall[:, t * 128:t * 128 + 64],
                                         sigo[:, 0:64], thc[:, 0:64])
                    mmgroup(ps_ob, 28, 32)
                    nc.vector.tensor_add(
                        gates_sb[:, 28:32, :],
                        ps_ob.rearrange("p (j n) -> p j n", n=JN)[:, :, 0:BS],
                        gxt_r[:, 28:32, t * BS:(t + 1) * BS])
                    nc.scalar.activation(sigo[:, 64:128], g2[:, 448:512], AF.Sigmoid,
                                         scale=1.0 / RSC)
                    nc.vector.tensor_mul(hall[:, t * 128 + 64:(t + 1) * 128],
                                         sigo[:, 64:128], thc[:, 64:128])
                if t == 0:
                    thc = elem.tile([128, 128], _F32, tag="thc", name="thc")
                    nc.scalar.activation(thc[:], cn[:], AF.Tanh)
                    nc.vector.tensor_mul(hall[:, t * 128:(t + 1) * 128],
                                         sigo2[:], thc[:])
                c_prev = cn

            # ---------------- Phase 3: logits = H @ fcn_W.T + fcn_b -------
            # W_hh / Gx space is dead now; hand it to the fcn weight pipeline
            gxtp.release()
            whhp.release()

            hsrc_r = hall.rearrange("p (t g) -> p t g", g=128)
            hfp = tc.alloc_tile_pool(name="hfp", bufs=1)
            hf = []
            for k in range(KT):
                hfk = hfp.tile([128, T - 1, BS], _DT[fcn], name=f"hf{k}")
                nc.vector.tensor_copy(hfk[:], hsrc_r[:, 1:T, k * BS:(k + 1) * BS])
                hf.append(hfk.rearrange("p a b -> p (a b)"))

            # chunk groups per output DMA: big groups early (DMA efficiency),
            # small ones at the end (shorter un-overlapped tail)
            groups = [OG] * (NOG - 1) + [3, 3, 3]
            assert sum(groups) == NCH
            with (
                tc.tile_pool(name="fcnp", bufs=4) as fcnp,
                tc.tile_pool(name="outp", bufs=2) as outp,
            ):
                c = 0
                for gn in groups:
                    ot = outp.tile([128, OG, 4, ROWS], _DT[odt], tag="ot",
                                   name="ot")
                    c0 = c
                    for ci in range(gn):
                        if c < NPF:
                            wt = pf[c]
                        else:
                            wt = fcnp.tile([128, KT * 512], _DT[fcn], tag="fw",
                                           name="fw")
                            nc.sync.dma_start(wt[:], fcnw_d[c])
                        for mi in range(4):
                            vt = c * 4 + mi
                            if vt >= VT:
                                continue  # junk vocab pad; host never reads it
                            ps = psum.tile([128, ROWS], _F32, tag="ps",
                                           name="psf", bufs=2)
                            for k in range(KT):
                                nc.tensor.matmul(
                                    ps[:],
                                    _mm_ap(wt[:, k * 512 + mi * 128:
                                              k * 512 + mi * 128 + 128], fcn),
                                    _mm_ap(hf[k], fcn),
                                    start=(k == 0), stop=(k == KT - 1))
                            nc.scalar.activation(ot[:, ci, mi, :], ps[:],
                                                 AF.Identity,
                                                 bias=fb_sb[:, vt:vt + 1])
                        c += 1
                    span = gn * 4 * ROWS
                    nc.sync.dma_start(
                        out_d[:, c0 * 4 * ROWS:c0 * 4 * ROWS + span],
                        ot.rearrange("p a b c -> p (a b c)")[:, 0:span])
            hfp.release()
            pfp.release()

    nc.finalize()
    return nc


def _prep_shared(W_ih, W_hh, b_ih, b_hh, fcn_W, fcn_b, cfg):
    """Host-side layout transforms (no FLOPs beyond the bias sum)."""
    perm = np.concatenate([np.arange(s * 128, (s + 1) * 128) for s in PERM_SRC])
    p1np, recnp, fcnnp = _NPDT[cfg["p1"]], _NPDT[cfg["rec"]], _NPDT[cfg["fcn"]]
    rsc = _RSC if cfg["rec"] == "f8" else 1.0

    wihT = np.zeros((H, 4 * H), np.float32)
    wihT[:XDIM, :] = np.asarray(W_ih, np.float32)[perm].T
    wih_t = np.ascontiguousarray(
        wihT.reshape(KT, 128, 4 * H).transpose(1, 0, 2).reshape(128, KT * 4 * H)
    ).astype(p1np)

    whhT = np.asarray(W_hh, np.float32)[perm].T * rsc  # [H, 4H]
    if cfg.get("dr", 0) and cfg["rec"] == "f8":
        # DoubleRow layout [p, kpair, o, gate]: the o (k-tile-pair lane) dim
        # must be LAST in the matmul APs with an element step %16 == 0
        whh_t = np.ascontiguousarray(
            whhT.reshape(KT // 2, 2, 128, 4 * H).transpose(2, 0, 1, 3)
            .reshape(128, KT * 4 * H)).astype(recnp)
    else:
        whh_t = np.ascontiguousarray(
            whhT.reshape(KT, 128, 4 * H).transpose(1, 0, 2).reshape(128, KT * 4 * H)
        ).astype(recnp)

    fw = np.zeros((VP, H), np.float32)
    fw[:V] = np.asarray(fcn_W, np.float32)
    fcnw_t = np.ascontiguousarray(
        fw.T.reshape(KT, 128, NCH, 512).transpose(2, 1, 0, 3).reshape(NCH, 128, KT * 512)
    ).astype(fcnnp)

    bsum = (np.asarray(b_ih, np.float32) + np.asarray(b_hh, np.float32))[perm] * rsc
    bsum_t = np.ascontiguousarray(bsum.reshape(GS, 128).T)
    fbp = np.zeros(VP, np.float32)
    fbp[:V] = np.asarray(fcn_b, np.float32)
    fb_t = np.ascontiguousarray(fbp.reshape(VTP, 128).T)
    return {"wih": wih_t, "whh": whh_t, "fcnw": fcnw_t,
            "bsum": bsum_t, "fb": fb_t}


def _prep_core(features, captions, emb_W, core, cfg):
    p1np = _NPDT[cfg["p1"]]
    sl = slice(core * BS, (core + 1) * BS)
    feats = np.asarray(features, np.float32)[sl]          # [16, ENC]
    caps = np.asarray(captions)[sl]                       # [16, T]
    embW = np.asarray(emb_W, np.float32)

    words = np.empty((BS, T, E), np.float32)
    words[:, 0, :] = embW[1]
    words[:, 1:, :] = embW[caps[:, :-1]]

    xpad = np.zeros((H, TB), np.float32)                  # [1024, 400]
    xpad[:E] = words.transpose(2, 1, 0).reshape(E, TB)    # (e, t, b)
    xpad[E:XDIM] = np.broadcast_to(
        feats.T[:, None, :], (ENC, T, BS)).reshape(ENC, TB)
    xT_t = np.ascontiguousarray(
        xpad.reshape(KT, 128, TB).transpose(1, 0, 2).reshape(128, KT * TB)
    ).astype(p1np)
    return {"xT": xT_t}


_BUILT = {}


def kernel(features, captions, emb_W, W_ih, W_hh, b_ih, b_hh,
           enc_W, enc_b, dec_W, dec_b, full_W, full_b, fcn_W, fcn_b,
           _cfg=None, _trace=False):
    cfg = dict(CFG if _cfg is None else _cfg)
    key = tuple(sorted(cfg.items()))
    if key not in _BUILT:
        _BUILT[key] = build_nc(cfg)
    nc = _BUILT[key]

    shared = _prep_shared(W_ih, W_hh, b_ih, b_hh, fcn_W, fcn_b, cfg)
    in_maps = []
    for c in range(NCORES):
        m = dict(shared)
        m.update(_prep_core(features, captions, emb_W, c, cfg))
        in_maps.append(m)

    res = run_bass_kernel_spmd(nc, in_maps, list(range(NCORES)), trace=_trace)

    out = np.empty((B, T - 1, V), np.float32)
    for c in range(NCORES):
        o = np.asarray(res.results[c]["out"], np.float32)
        o = o.reshape(128, NCH, 4, T - 1, BS)
        # vocab v = (ch*4+mi)*128 + p ; want [b, t, v]
        o = o.transpose(3, 4, 1, 2, 0).reshape(T - 1, BS, VP)[:, :, :V]
        out[c * BS:(c + 1) * BS] = o.transpose(1, 0, 2)
    kernel._last_result = res
    return out



# revision 46
# speedup vs baseline: 1.1073x; 1.1073x over previous
"""Trainium2 Bass kernel for DecoderRNNWithAttention (teacher-forced LSTM decoder).

Key mathematical simplification: the attention block is an exact no-op.
The encoder output has a single spatial position, so softmax over that
axis is exactly 1.0 and context == features, independent of h. Hence:
  - the enc/dec/full attention projections never affect the output;
  - the input-side gate contributions Gx = X @ W_ih.T + (b_ih + b_hh)
    can be precomputed for all T steps in one batched matmul
    (X_t = [word_t ; features]);
  - the serial recurrence is only gates_t = Gx_t + h_t @ W_hh.T plus the
    LSTM elementwise cell; logits_t = h_{t+1} @ fcn_W.T + fcn_b.

Sharding: pure data-parallel over batch. 8 cores x 16 rows, no collectives.

Device layouts (all "transposed" so the partition dim is the feature dim):
  - gate dim 4H split into 32 slices of 128, permuted [i f o g] so one
    sigmoid covers cols 0:384 and one tanh covers cols 384:512 of the
    per-step [128, 512] gate tile (cols = slice-block * 16 batch).
  - h state history Hall[128, t*128 + k*16 + b] (k = H-tile), written
    once per step as one [128, 128] tile; doubles as matmul rhs slices.
  - vocab projection: out.T tiles [V-tile 128, 384 rows], rows = (t, b).
"""

import numpy as np
import ml_dtypes

import concourse.bacc as bacc
import concourse.mybir as mybir
import concourse.tile as tile
from concourse.bass_utils import run_bass_kernel_spmd

B, T, E, H, V, ENC = 128, 25, 512, 1024, 32000, 400
NCORES = 8
BS = B // NCORES          # 16 batch rows per core
TB = T * BS               # 400 = matmul N for phase 1
ROWS = (T - 1) * BS       # 384 = matmul N for the vocab projection
KT = H // 128             # 8 K-tiles
GS = 4 * H // 128         # 32 gate slices
VT = V // 128             # 250 vocab tiles
NCH = 63                  # fcn weight chunks (512 vocab cols each)
VP = NCH * 512            # 32256 = vocab padded to whole chunks
VTP = VP // 128           # 252 padded vocab tiles
XDIM = E + ENC            # 912, padded to 1024

# torch LSTMCell gate order is [i f g o] — kept as-is: the o-gate stream
# lands last so the c-path (needs i,f,g) overlaps the o matmuls, and the
# per-group adds/activations pipeline with the stream.
PERM_SRC = list(range(0, 32))

# dtype config: 'f32' exact, 'f32r' tf32-like at bf16 speed (N>=256),
# 'bf16' fastest. Storage for f32r is fp32 + bitcast at the matmul.
CFG = {
    "p1": "bf16",    # phase-1 (Gx) matmul dtype
    "rec": "f8",     # recurrence (W_hh) matmul dtype (fp8 e4m3, x256 prescale)
    "fcn": "bf16",   # vocab projection matmul dtype
    "vchunk": 512,  # fcn weight streaming chunk (vocab cols per DMA)
    "jn": 64,        # recurrence matmul free dim (>=BS; junk cols keep HAM warm)
    "npf": 4,        # fcn weight chunks prefetched into SBUF during P2
    "og": 9,         # fcn chunks per grouped output DMA (63 = 7 * 9)
    "odt": "f16",    # output store dtype (f16 halves the logits DMA bytes)
    "dr": 0,         # fp8 DoubleRow recurrence (measured net loss; keep off)
}

_F32 = mybir.dt.float32
_DT = {"f32": mybir.dt.float32, "f32r": mybir.dt.float32r, "bf16": mybir.dt.bfloat16,
       "f16": mybir.dt.float16, "f8": mybir.dt.float8e4}
_NPDT = {"f32": np.float32, "f32r": np.float32, "bf16": ml_dtypes.bfloat16,
         "f16": np.float16, "f8": ml_dtypes.float8_e4m3}
# fp8 W_hh is pre-scaled so ~all weights land in e4m3's normal range; the
# inverse scale folds into the gate activations (out = f(in * scale)).
_RSC = 256.0


def _mm_ap(ap, kind):
    """Tiles carrying f32r data are declared float32r; nothing to do."""
    return ap


def build_nc(cfg=CFG):
    AF = mybir.ActivationFunctionType
    p1, rec, fcn = cfg["p1"], cfg["rec"], cfg["fcn"]
    VC = cfg["vchunk"]
    odt = cfg.get("odt", "f16")
    NPF = cfg.get("npf", 8)
    OG = cfg.get("og", 9)
    NOG = NCH // OG
    assert NOG * OG == NCH
    # gates carry an extra factor RSC when W_hh is fp8-prescaled
    RSC = _RSC if rec == "f8" else 1.0
    DR = bool(cfg.get("dr", 0)) and rec == "f8"   # fp8 DoubleRow recurrence
    hdt = ("f8" if DR else "bf16") if rec == "f8" else rec  # h storage dtype

    JN = cfg.get("jn", BS)
    # hall row stride: pad so k-slice reads with JN junk columns never cross
    # into the next step's row (which is now written mid-stream in halves)
    HROW = 128 if JN <= BS else 128 + (JN - BS + 15) // 16 * 16

    nc = bacc.Bacc()
    xT_d = nc.dram_tensor("xT", [128, KT * TB], _DT[p1], kind="ExternalInput")
    wih_d = nc.dram_tensor("wih", [128, KT * 4 * H], _DT[p1], kind="ExternalInput")
    whh_d = nc.dram_tensor("whh", [128, KT * 4 * H], _DT[rec], kind="ExternalInput")
    fcnw_d = nc.dram_tensor("fcnw", [NCH, 128, KT * 512], _DT[fcn], kind="ExternalInput")
    bsum_d = nc.dram_tensor("bsum", [128, GS], _F32, kind="ExternalInput")
    fb_d = nc.dram_tensor("fb", [128, VTP], _F32, kind="ExternalInput")
    out_d = nc.dram_tensor("out", [128, NCH * 4 * ROWS], _DT[odt],
                           kind="ExternalOutput")

    with tile.TileContext(nc) as tc:
        with (
            tc.tile_pool(name="pers", bufs=1) as pers,
            tc.tile_pool(name="psum", bufs=4, space="PSUM") as psum,
            tc.tile_pool(name="elem", bufs=2) as elem,
        ):
            hall = pers.tile([128, T * HROW], _DT[hdt])
            xt_sb = pers.tile([128, KT * TB], _DT[p1])
            bsum_sb = pers.tile([128, GS], _F32)
            fb_sb = pers.tile([128, VTP], _F32)
            # fcn weight prefetch pool: allocated BELOW whhp/gxtp on the SBUF
            # stack so it survives their (LIFO) release into phase 3. The
            # prefetch DMAs depend only on DRAM, so they drain during the
            # recurrence while the DMA queues are otherwise idle.
            pfp = tc.alloc_tile_pool(name="pfp", bufs=1)
            pf = [pfp.tile([128, KT * 512], _DT[fcn], name=f"pf{c}")
                  for c in range(NPF)]
            # W_hh and Gx live only through the recurrence; own pools so the
            # space can be released to the fcn weight-streaming pool (LIFO).
            whhp = tc.alloc_tile_pool(name="whhp", bufs=1)
            gxtp = tc.alloc_tile_pool(name="gxtp", bufs=1)
            whh_sb = whhp.tile([128, KT * 4 * H], _DT[rec], name="whh_sb")
            gxt = gxtp.tile([128, GS * TB], _F32, name="gxt")

            nc.sync.dma_start(xt_sb[:], xT_d[:])
            nc.sync.dma_start(bsum_sb[:], bsum_d[:])
            nc.sync.dma_start(fb_sb[:], fb_d[:])
            nc.gpsimd.memset(hall[:], 0.0)

            # ---------------- Phase 1: Gx = X @ W_ih.T + (b_ih + b_hh) ----
            with tc.tile_pool(name="wihp", bufs=2) as wihp:
                for quarter in range(4):
                    wih_sb = wihp.tile([128, KT * 1024], _DT[p1], tag="wih")
                    for k in range(KT):
                        nc.sync.dma_start(
                            wih_sb[:, k * 1024:(k + 1) * 1024],
                            wih_d[:, k * 4096 + quarter * 1024:
                                  k * 4096 + quarter * 1024 + 1024])
                    for jj in range(8):
                        j = quarter * 8 + jj
                        ps = psum.tile([128, TB], _F32, tag="ps", name="ps", bufs=2)
                        for k in range(KT):
                            nc.tensor.matmul(
                                ps[:],
                                _mm_ap(wih_sb[:, k * 1024 + jj * 128:
                                              k * 1024 + jj * 128 + 128], p1),
                                _mm_ap(xt_sb[:, k * TB:(k + 1) * TB], p1),
                                start=(k == 0), stop=(k == KT - 1))
                        nc.scalar.activation(
                            gxt[:, j * TB:(j + 1) * TB], ps[:], AF.Identity,
                            bias=bsum_sb[:, j:j + 1], scale=RSC)

            # W_hh load ordered after phase-1 inputs so phase 1 starts early
            for k in range(KT):
                nc.sync.dma_start(whh_sb[:, k * 4096:(k + 1) * 4096],
                                  whh_d[:, k * 4096:(k + 1) * 4096])
            for c in range(NPF):
                # dummy write gives the prefetch DMA a dep on the last P1
                # activation, keeping its HBM traffic out of P1's window
                # (it drains during the recurrence instead)
                nc.vector.tensor_copy(pf[c][:, 0:1], gxt[:, GS * TB - 1:GS * TB])
                nc.sync.dma_start(pf[c][:], fcnw_d[c])

            # ---------------- Phase 2: LSTM recurrence --------------------
            # gxt viewed as [128, slice j, t, b]
            gxt_r = gxt.rearrange("p (j t b) -> p j (t b)", j=GS, t=T, b=BS)

            c_prev = None
            for t in range(T):
                if t == 0:
                    # gate order [i f g o]; c0 == 0 so f is unused
                    gates_src = gxt_r[:, :, 0:BS]  # [128, 32, 16] strided
                    sig_sb = elem.tile([128, 8, BS], _F32, tag="sig0", name="sg0")
                    nc.scalar.activation(sig_sb[:], gates_src[:, 0:8, :], AF.Sigmoid,
                                         scale=1.0 / RSC)
                    tg = elem.tile([128, 8, BS], _F32, tag="tg", name="tg")
                    nc.scalar.activation(tg[:], gates_src[:, 16:24, :], AF.Tanh,
                                         scale=1.0 / RSC)
                    sigo = elem.tile([128, 8, BS], _F32, tag="sigo", name="so0")
                    nc.scalar.activation(sigo[:], gates_src[:, 24:32, :], AF.Sigmoid,
                                         scale=1.0 / RSC)
                    sigo2 = sigo.rearrange("p a b -> p (a b)")
                    cn = elem.tile([128, 128], _F32, tag="c", name="cn")
                    nc.vector.tensor_mul(cn[:], sig_sb.rearrange("p a b -> p (a b)"),
                                         tg.rearrange("p a b -> p (a b)"))
                else:
                    # separate PSUM tiles per gate group so the adds/ACTs
                    # depend only on their own group's matmuls and pipeline
                    # with the still-running stream (i,f first, o last)
                    ps_if = psum.tile([128, 16 * JN], _F32, tag="psif",
                                      name="psif", bufs=1 if JN > 32 else 2)
                    ps_gg = psum.tile([128, 8 * JN], _F32, tag="psgg",
                                      name="psgg", bufs=1 if JN > 32 else 2)
                    ps_oa = psum.tile([128, 4 * JN], _F32, tag="psoa",
                                      name="psoa", bufs=1)
                    ps_ob = psum.tile([128, 4 * JN], _F32, tag="psob",
                                      name="psob", bufs=1)

                    if DR:
                        def mmgroup(pst, j0, j1):
                            # fp8 DoubleRow: one matmul contracts a k-tile
                            # PAIR (256 rows); lhsT [128, 2, 128] interleaved
                            # on host, rhs [128, 2, JN] is just a strided view
                            # of hall's (k,b) column layout.
                            for j in range(j0, j1):
                                for a in range(KT // 2):
                                    nc.tensor.matmul(
                                        pst[:, (j - j0) * JN:(j - j0) * JN + JN],
                                        whh_sb[:, a * 8192:(a + 1) * 8192]
                                        .rearrange("p (o g) -> p o g", o=2)
                                        [:, :, j * 128:(j + 1) * 128],
                                        hall[:, (t - 1) * HROW + a * 32:
                                             (t - 1) * HROW + a * 32 + 2 * JN]
                                        .rearrange("p (o n) -> p o n", o=2),
                                        start=(a == 0), stop=(a == KT // 2 - 1),
                                        perf_mode=mybir.MatmulPerfMode.DoubleRow)
                    else:
                        def mmgroup(pst, j0, j1):
                            for j in range(j0, j1):
                                for k in range(KT):
                                    nc.tensor.matmul(
                                        pst[:, (j - j0) * JN:(j - j0) * JN + JN],
                                        _mm_ap(whh_sb[:, k * 4096 + j * 128:
                                                      k * 4096 + j * 128 + 128], rec),
                                        _mm_ap(hall[:, (t - 1) * HROW + k * BS:
                                                    (t - 1) * HROW + k * BS + JN], rec),
                                        start=(k == 0), stop=(k == KT - 1))

                    gates_sb = elem.tile([128, GS, BS], _F32, tag="gates", name="gts")
                    g2 = gates_sb.rearrange("p a b -> p (a b)")

                    mmgroup(ps_if, 0, 16)
                    nc.vector.tensor_add(
                        gates_sb[:, 0:16, :],
                        ps_if.rearrange("p (j n) -> p j n", n=JN)[:, :, 0:BS],
                        gxt_r[:, 0:16, t * BS:(t + 1) * BS])
                    sig_sb = elem.tile([128, 256], _F32, tag="sig", name="sig")
                    nc.scalar.activation(sig_sb[:], g2[:, 0:256], AF.Sigmoid,
                                         scale=1.0 / RSC)

                    mmgroup(ps_gg, 16, 24)
                    nc.vector.tensor_add(
                        gates_sb[:, 16:24, :],
                        ps_gg.rearrange("p (j n) -> p j n", n=JN)[:, :, 0:BS],
                        gxt_r[:, 16:24, t * BS:(t + 1) * BS])
                    tg = elem.tile([128, 128], _F32, tag="tg", name="tg")
                    nc.scalar.activation(tg[:], g2[:, 256:384], AF.Tanh,
                                         scale=1.0 / RSC)
                    cn = elem.tile([128, 128], _F32, tag="c", name="cn")
                    nc.vector.tensor_mul(cn[:], sig_sb[:, 128:256], c_prev[:])
                    t1 = elem.tile([128, 128], _F32, tag="t1", name="t1")
                    nc.vector.tensor_mul(t1[:], sig_sb[:, 0:128], tg[:])
                    nc.vector.tensor_add(cn[:], cn[:], t1[:])
                    # thc queued before sig_o on the (FIFO) scalar engine: its
                    # input is ready well before the o-gate matmuls finish
                    thc = elem.tile([128, 128], _F32, tag="thc", name="thc")
                    nc.scalar.activation(thc[:], cn[:], AF.Tanh)

                    # o-group in two halves: half A's add/sigmoid/h-write hide
                    # under half B's matmul stream, shortening the serial tail
                    sigo = elem.tile([128, 128], _F32, tag="sigo", name="sgo")
                    mmgroup(ps_oa, 24, 28)
                    nc.vector.tensor_add(
                        gates_sb[:, 24:28, :],
                        ps_oa.rearrange("p (j n) -> p j n", n=JN)[:, :, 0:BS],
                        gxt_r[:, 24:28, t * BS:(t + 1) * BS])
                    nc.scalar.activation(sigo[:, 0:64], g2[:, 384:448], AF.Sigmoid,
                                         scale=1.0 / RSC)
                    nc.vector.tensor_mul(hall[:, t * HROW:t * HROW + 64],
                                         sigo[:, 0:64], thc[:, 0:64])
                    mmgroup(ps_ob, 28, 32)
                    nc.vector.tensor_add(
                        gates_sb[:, 28:32, :],
                        ps_ob.rearrange("p (j n) -> p j n", n=JN)[:, :, 0:BS],
                        gxt_r[:, 28:32, t * BS:(t + 1) * BS])
                    nc.scalar.activation(sigo[:, 64:128], g2[:, 448:512], AF.Sigmoid,
                                         scale=1.0 / RSC)
                    nc.vector.tensor_mul(hall[:, t * HROW + 64:t * HROW + 128],
                                         sigo[:, 64:128], thc[:, 64:128])
                if t == 0:
                    thc = elem.tile([128, 128], _F32, tag="thc", name="thc")
                    nc.scalar.activation(thc[:], cn[:], AF.Tanh)
                    nc.vector.tensor_mul(hall[:, t * HROW:t * HROW + 128],
                                         sigo2[:], thc[:])
                c_prev = cn

            # ---------------- Phase 3: logits = H @ fcn_W.T + fcn_b -------
            # W_hh / Gx space is dead now; hand it to the fcn weight pipeline
            gxtp.release()
            whhp.release()

            hsrc_r = hall.rearrange("p (t g) -> p t g", g=HROW)
            hfp = tc.alloc_tile_pool(name="hfp", bufs=1)
            hf = []
            for k in range(KT):
                hfk = hfp.tile([128, T - 1, BS], _DT[fcn], name=f"hf{k}")
                nc.vector.tensor_copy(hfk[:], hsrc_r[:, 1:T, k * BS:(k + 1) * BS])
                hf.append(hfk.rearrange("p a b -> p (a b)"))

            # chunk groups per output DMA: big groups early (DMA efficiency),
            # small ones at the end (shorter un-overlapped tail)
            groups = [OG] * (NOG - 1) + [3, 3, 3]
            assert sum(groups) == NCH
            with (
                tc.tile_pool(name="fcnp", bufs=4) as fcnp,
                tc.tile_pool(name="outp", bufs=2) as outp,
            ):
                c = 0
                for gn in groups:
                    ot = outp.tile([128, OG, 4, ROWS], _DT[odt], tag="ot",
                                   name="ot")
                    c0 = c
                    for ci in range(gn):
                        if c < NPF:
                            wt = pf[c]
                        else:
                            wt = fcnp.tile([128, KT * 512], _DT[fcn], tag="fw",
                                           name="fw")
                            nc.sync.dma_start(wt[:], fcnw_d[c])
                        for mi in range(4):
                            vt = c * 4 + mi
                            if vt >= VT:
                                continue  # junk vocab pad; host never reads it
                            ps = psum.tile([128, ROWS], _F32, tag="ps",
                                           name="psf", bufs=2)
                            for k in range(KT):
                                nc.tensor.matmul(
                                    ps[:],
                                    _mm_ap(wt[:, k * 512 + mi * 128:
                                              k * 512 + mi * 128 + 128], fcn),
                                    _mm_ap(hf[k], fcn),
                                    start=(k == 0), stop=(k == KT - 1))
                            nc.scalar.activation(ot[:, ci, mi, :], ps[:],
                                                 AF.Identity,
                                                 bias=fb_sb[:, vt:vt + 1])
                        c += 1
                    span = gn * 4 * ROWS
                    nc.sync.dma_start(
                        out_d[:, c0 * 4 * ROWS:c0 * 4 * ROWS + span],
                        ot.rearrange("p a b c -> p (a b c)")[:, 0:span])
            hfp.release()
            pfp.release()

    nc.finalize()
    return nc


def _prep_shared(W_ih, W_hh, b_ih, b_hh, fcn_W, fcn_b, cfg):
    """Host-side layout transforms (no FLOPs beyond the bias sum)."""
    perm = np.concatenate([np.arange(s * 128, (s + 1) * 128) for s in PERM_SRC])
    p1np, recnp, fcnnp = _NPDT[cfg["p1"]], _NPDT[cfg["rec"]], _NPDT[cfg["fcn"]]
    rsc = _RSC if cfg["rec"] == "f8" else 1.0

    wihT = np.zeros((H, 4 * H), np.float32)
    wihT[:XDIM, :] = np.asarray(W_ih, np.float32)[perm].T
    wih_t = np.ascontiguousarray(
        wihT.reshape(KT, 128, 4 * H).transpose(1, 0, 2).reshape(128, KT * 4 * H)
    ).astype(p1np)

    whhT = np.asarray(W_hh, np.float32)[perm].T * rsc  # [H, 4H]
    if cfg.get("dr", 0) and cfg["rec"] == "f8":
        # DoubleRow layout [p, kpair, o, gate]: the o (k-tile-pair lane) dim
        # must be LAST in the matmul APs with an element step %16 == 0
        whh_t = np.ascontiguousarray(
            whhT.reshape(KT // 2, 2, 128, 4 * H).transpose(2, 0, 1, 3)
            .reshape(128, KT * 4 * H)).astype(recnp)
    else:
        whh_t = np.ascontiguousarray(
            whhT.reshape(KT, 128, 4 * H).transpose(1, 0, 2).reshape(128, KT * 4 * H)
        ).astype(recnp)

    fw = np.zeros((VP, H), np.float32)
    fw[:V] = np.asarray(fcn_W, np.float32)
    fcnw_t = np.ascontiguousarray(
        fw.T.reshape(KT, 128, NCH, 512).transpose(2, 1, 0, 3).reshape(NCH, 128, KT * 512)
    ).astype(fcnnp)

    bsum = (np.asarray(b_ih, np.float32) + np.asarray(b_hh, np.float32))[perm] * rsc
    bsum_t = np.ascontiguousarray(bsum.reshape(GS, 128).T)
    fbp = np.zeros(VP, np.float32)
    fbp[:V] = np.asarray(fcn_b, np.float32)
    fb_t = np.ascontiguousarray(fbp.reshape(VTP, 128).T)
    return {"wih": wih_t, "whh": whh_t, "fcnw": fcnw_t,
            "bsum": bsum_t, "fb": fb_t}


def _prep_core(features, captions, emb_W, core, cfg):
    p1np = _NPDT[cfg["p1"]]
    sl = slice(core * BS, (core + 1) * BS)
    feats = np.asarray(features, np.float32)[sl]          # [16, ENC]
    caps = np.asarray(captions)[sl]                       # [16, T]
    embW = np.asarray(emb_W, np.float32)

    words = np.empty((BS, T, E), np.float32)
    words[:, 0, :] = embW[1]
    words[:, 1:, :] = embW[caps[:, :-1]]

    xpad = np.zeros((H, TB), np.float32)                  # [1024, 400]
    xpad[:E] = words.transpose(2, 1, 0).reshape(E, TB)    # (e, t, b)
    xpad[E:XDIM] = np.broadcast_to(
        feats.T[:, None, :], (ENC, T, BS)).reshape(ENC, TB)
    xT_t = np.ascontiguousarray(
        xpad.reshape(KT, 128, TB).transpose(1, 0, 2).reshape(128, KT * TB)
    ).astype(p1np)
    return {"xT": xT_t}


_BUILT = {}


def kernel(features, captions, emb_W, W_ih, W_hh, b_ih, b_hh,
           enc_W, enc_b, dec_W, dec_b, full_W, full_b, fcn_W, fcn_b,
           _cfg=None, _trace=False):
    cfg = dict(CFG if _cfg is None else _cfg)
    key = tuple(sorted(cfg.items()))
    if key not in _BUILT:
        _BUILT[key] = build_nc(cfg)
    nc = _BUILT[key]

    shared = _prep_shared(W_ih, W_hh, b_ih, b_hh, fcn_W, fcn_b, cfg)
    in_maps = []
    for c in range(NCORES):
        m = dict(shared)
        m.update(_prep_core(features, captions, emb_W, c, cfg))
        in_maps.append(m)

    res = run_bass_kernel_spmd(nc, in_maps, list(range(NCORES)), trace=_trace)

    out = np.empty((B, T - 1, V), np.float32)
    for c in range(NCORES):
        o = np.asarray(res.results[c]["out"], np.float32)
        o = o.reshape(128, NCH, 4, T - 1, BS)
        # vocab v = (ch*4+mi)*128 + p ; want [b, t, v]
        o = o.transpose(3, 4, 1, 2, 0).reshape(T - 1, BS, VP)[:, :, :V]
        out[c * BS:(c + 1) * BS] = o.transpose(1, 0, 2)
    kernel._last_result = res
    return out

